# revision 63
# baseline (speedup 1.0000x reference)
"""DFlashAttention kernel for Trainium2, tensor-parallel across 8 NeuronCores.

Sharding: Megatron-style head parallelism. Core c owns KV head c and Q heads
4c..4c+3 (matches repeat_interleave grouping), i.e. Wq rows [512c, 512c+512),
Wk/Wv rows [128c, 128c+128), Wo columns [512c, 512c+512). Each core computes a
partial output [QL, H]; the host sums the 8 partials (row-parallel Wo).

v2 layout notes:
  - weights + cos/sin persist in SBUF across body reps (loaded once)
  - head-dim rows of Wq/Wk (and cos/sin) are permuted host-side so that the
    RoPE rotate_half partner lives in the same 32-partition quadrant; the
    swap is then a single DVE stream_shuffle (scores are invariant to a
    shared permutation of the contraction dim)
  - phase A: 256-wide position chunks; K/V noise+ctx projected by one
    N=512 matmul from interleaved [hs|ts] staging tiles; Q in 4 half banks
    (6 PSUM banks total, so phase A overlaps the previous rep's B/C)
  - phase B: S^T scores per 128-k tile, exp on ACT into a rolling 8-slot
    bf16 buffer, PV + ones-rowsum accumulated on PE; reciprocal broadcast
    via PE; attn^T normalized into a per-rep bf16 `at` buffer
  - phase C interleaved per q-block: out = at^T @ Wo, DVE 2x copies to
    bf16, 64 big output DMAs; host sums bf16 partials in f32
"""

import math
from contextlib import ExitStack

import ml_dtypes
import numpy as np

import concourse.bass as bass
import concourse.bacc as bacc
import concourse.mybir as mybir
import concourse.tile as tile
from concourse.bass_utils import run_bass_kernel_spmd

F32 = mybir.dt.float32
F32R = mybir.dt.float32r
BF16 = mybir.dt.bfloat16
AF = mybir.ActivationFunctionType
ALU = mybir.AluOpType

# Full-problem dims (hardcoded per spec)
B, QL, CTX, H = 1, 2048, 2048, 4096
NH, NKV, HD = 32, 8, 128
NCORES = 8
HPC = NH // NKV  # 4 q-heads per core (one KV head per core)

QC = 256            # phase A position chunk
QB = 512            # phase B q-block
EG = 8              # e-tiles per staging group

SWAP_MASK = [(i + 16) % 32 for i in range(32)]


def head_perm():
    """Permutation of the 128 head dims: rotate_half partner in-quadrant."""
    pi = np.zeros(128, dtype=np.int64)
    for q in range(4):
        for i in range(16):
            pi[32 * q + i] = 16 * q + i
            pi[32 * q + 16 + i] = 64 + 16 * q + i
    return pi


def build_program(ql=QL, ctx_len=CTX, h=H, trace_sim=False, phases="ABC", body_reps=1):
    """Build the per-core Bass program (SPMD: same program, per-core shards)."""
    s = ql + ctx_len          # total kv length
    et = h // 128             # e-tiles (contraction tiles for projections)
    kt = s // 128             # k-tiles in attention
    nch = ql // QC            # phase A chunks
    assert ctx_len == ql, "phase A chunking assumes ctx_len == ql"
    nqb = ql // QB
    ngr = et // EG            # staging groups per chunk
    scale = 1.0 / math.sqrt(HD)
    DQ = HPC * HD             # 512: per-core q-head dim

    nc = bacc.Bacc("TRN2", target_bir_lowering=False, debug=False)

    def din(name, shape, dt_=F32):
        return nc.dram_tensor(name, shape, dt_, kind="ExternalInput").ap()

    hiddenT = din("hiddenT", [h, ql], BF16)       # hidden_states[0].T
    targetT = din("targetT", [h, ctx_len], BF16)  # target_hidden[0].T
    cosP = din("cosP", [HD, s], BF16)             # permuted cos[0].T
    sinP = din("sinP", [HD, s], BF16)             # permuted, sign-folded sin
    wqT = din("wqT", [h, DQ], BF16)               # Wq[shard].T, cols permuted
    wkT = din("wkT", [h, HD], BF16)               # cols permuted
    wvT = din("wvT", [h, HD], BF16)
    woT = din("woT", [DQ, h], BF16)               # Wo[:, shard].T
    ones_d = din("ones", [128, 128], F32R)
    onesb_d = din("ones_bf", [128, 1], BF16)
    identb_d = din("identb", [128, 128], BF16)
    out_d = nc.dram_tensor("out", [ql, h], BF16, kind="ExternalOutput").ap()

    with tile.TileContext(nc, trace_sim=trace_sim) as tc, ExitStack() as ctx:
        persist = ctx.enter_context(tc.tile_pool(name="persist", bufs=1))
        big = ctx.enter_context(tc.tile_pool(name="big", bufs=2))
        atp = ctx.enter_context(tc.tile_pool(name="atp", bufs=1))
        stage = ctx.enter_context(tc.tile_pool(name="stage", bufs=2))
        sbp = ctx.enter_context(tc.tile_pool(name="sbp", bufs=1))
        ps = ctx.enter_context(
            tc.tile_pool(name="ps", bufs=8, space=bass.MemorySpace.PSUM)
        )

        # ---- persistent (once per NEFF): weights, rotary tables, consts ----
        cos_sb = persist.tile([128, s], BF16, tag="cos")
        sin_sb = persist.tile([128, s], BF16, tag="sin")
        nc.sync.dma_start(cos_sb[:], cosP[:])
        nc.sync.dma_start(sin_sb[:], sinP[:])
        wq_sb = persist.tile([128, et, DQ], BF16, tag="wq")
        wk_sb = persist.tile([128, et, HD], BF16, tag="wk")
        wv_sb = persist.tile([128, et, HD], BF16, tag="wv")
        nc.sync.dma_start(wq_sb[:], wqT.rearrange("(e p) d -> p e d", p=128))
        nc.sync.dma_start(wk_sb[:], wkT.rearrange("(e p) d -> p e d", p=128))
        nc.sync.dma_start(wv_sb[:], wvT.rearrange("(e p) d -> p e d", p=128))
        wo_sb = persist.tile([128, HPC, h], BF16, tag="wo")
        nc.sync.dma_start(wo_sb[:], woT.rearrange("(t p) o -> p t o", p=128))
        ones_sb = persist.tile([128, 128], F32R, tag="ones")
        onesb_sb = persist.tile([128, 1], BF16, tag="onesb")
        identb_sb = persist.tile([128, 128], BF16, tag="identb")
        nc.sync.dma_start(ones_sb[:], ones_d[:])
        nc.sync.dma_start(onesb_sb[:], onesb_d[:])
        nc.sync.dma_start(identb_sb[:], identb_d[:])

        hiddenR = hiddenT.rearrange("(e p) q -> p e q", p=128)
        targetR = targetT.rearrange("(e p) q -> p e q", p=128)

        for _rep in range(body_reps):
            qr = atp.tile([128, HPC, ql], BF16, tag="qr")   # [d, h, q]
            kr = big.tile([128, s], BF16, tag="kr")          # [d, k]
            v_sb = big.tile([128, kt, HD], BF16, tag="v")    # [k%128, kt, d]
            at = atp.tile([128, HPC, ql], BF16, tag="at")    # [d, h, q]

            # ---------------- Phase A: projections + RoPE + V transpose ----
            def rope(ps_slice, cosl, sinl, dst):
                raw = sbp.tile([128, QC], BF16, tag="rraw", bufs=3)
                nc.scalar.copy(raw[:], ps_slice)
                swp = sbp.tile([128, QC], BF16, tag="rswp", bufs=2)
                nc.vector.stream_shuffle(swp[:], raw[:], SWAP_MASK)
                t1 = sbp.tile([128, QC], BF16, tag="rt1", bufs=2)
                nc.vector.tensor_tensor(t1[:], raw[:], cosl, ALU.mult)
                t2 = sbp.tile([128, QC], BF16, tag="rt2", bufs=2)
                nc.vector.tensor_tensor(t2[:], swp[:], sinl, ALU.mult)
                nc.vector.tensor_tensor(dst, t1[:], t2[:], ALU.add)

            for c in range(nch):
                q0 = c * QC
                psq = [ps.tile([128, QC], F32, tag="ps", name=f"psq{c}_{i}")
                       for i in range(HPC)]
                psk = ps.tile([128, 2 * QC], F32, tag="ps")  # [kn | kc]
                psv = ps.tile([128, 2 * QC], F32, tag="ps")  # [vn | vc]

                for g in range(ngr):
                    hts = stage.tile([128, EG, 2 * QC], BF16, tag="hts")
                    nc.sync.dma_start(
                        hts[:, :, 0:QC],
                        hiddenR[:, g * EG:(g + 1) * EG, q0:q0 + QC],
                    )
                    nc.sync.dma_start(
                        hts[:, :, QC:2 * QC],
                        targetR[:, g * EG:(g + 1) * EG, q0:q0 + QC],
                    )
                    for el in range(EG):
                        e = g * EG + el
                        st = dict(start=(e == 0), stop=(e == et - 1))
                        nc.tensor.matmul(
                            psk[:], wk_sb[:, e, :], hts[:, el, :], **st
                        )
                        nc.tensor.matmul(
                            psv[:], wv_sb[:, e, :], hts[:, el, :], **st
                        )
                        for hh in range(HPC):
                            nc.tensor.matmul(
                                psq[hh][:],
                                wq_sb[:, e, hh * 128:hh * 128 + 128],
                                hts[:, el, 0:QC],
                                **st,
                            )

                cn = cos_sb[:, ctx_len + q0:ctx_len + q0 + QC]
                sn = sin_sb[:, ctx_len + q0:ctx_len + q0 + QC]
                cc = cos_sb[:, q0:q0 + QC]
                sc = sin_sb[:, q0:q0 + QC]

                for hh in range(HPC):
                    rope(psq[hh][:], cn, sn, qr[:, hh, q0:q0 + QC])
                rope(psk[:, 0:QC], cn, sn,
                     kr[:, ctx_len + q0:ctx_len + q0 + QC])
                rope(psk[:, QC:2 * QC], cc, sc, kr[:, q0:q0 + QC])

                # V: [d, pos] -> PE transpose -> k-major bf16
                vd = sbp.tile([128, 2 * QC], BF16, tag="vd", bufs=2)
                nc.scalar.copy(vd[:], psv[:])
                pst = ps.tile([128, 4 * 128], BF16, tag="ps")
                for i in range(4):
                    nc.tensor.transpose(
                        pst[:, i * 128:i * 128 + 128],
                        vd[:, i * 128:i * 128 + 128],
                        identb_sb[:],
                    )
                jn = (ctx_len + q0) // 128    # noise tiles (from vd cols 0:256)
                jc = q0 // 128                # ctx tiles (from vd cols 256:512)
                nc.vector.tensor_copy(v_sb[:, jn:jn + 2, :], pst[:, 0:256])
                nc.vector.tensor_copy(v_sb[:, jc:jc + 2, :], pst[:, 256:512])

            # ------------- Phase B + C interleaved per q-block -------------
            for qb in range(nqb):
                qs0 = qb * QB
                for hh in range(HPC):
                    psat = ps.tile([128, QB], F32, tag="ps")
                    psrs = ps.tile([1, QB], F32, tag="ps")
                    exps = []
                    LOOKAHEAD = 2
                    ng = kt // 4   # rowsum groups (4:1 DVE tree per group)

                    def emit_scores(j):
                        pss = ps.tile([128, QB], F32, tag="ps",
                                      name=f"pss{qb}_{hh}_{j}")
                        nc.tensor.matmul(
                            pss[:],
                            kr[:, j * 128:j * 128 + 128],
                            qr[:, hh, qs0:qs0 + QB],
                            start=True,
                            stop=True,
                        )
                        ex = sbp.tile([128, QB], BF16, tag="expst", bufs=8)
                        nc.scalar.activation(ex[:], pss[:], AF.Exp, scale=scale)
                        exps.append(ex)

                    for j in range(min(LOOKAHEAD, kt)):
                        emit_scores(j)
                    for j in range(kt):
                        if j + LOOKAHEAD < kt:
                            emit_scores(j + LOOKAHEAD)
                        nc.tensor.matmul(
                            psat[:], v_sb[:, j, :], exps[j][:],
                            start=(j == 0), stop=(j == kt - 1),
                        )
                        if j % 4 == 3:
                            # rowsum offload: 4:1 bf16 tree on DVE, then one
                            # ones-matmul per group of 4 k-tiles
                            g = j // 4
                            a01 = sbp.tile([128, QB], BF16, tag="ra", bufs=2)
                            nc.vector.tensor_tensor(
                                a01[:], exps[j - 3][:], exps[j - 2][:], ALU.add)
                            a23 = sbp.tile([128, QB], BF16, tag="rb", bufs=2)
                            nc.vector.tensor_tensor(
                                a23[:], exps[j - 1][:], exps[j][:], ALU.add)
                            r4 = sbp.tile([128, QB], BF16, tag="rc", bufs=2)
                            nc.vector.tensor_tensor(
                                r4[:], a01[:], a23[:], ALU.add)
                            nc.tensor.matmul(
                                psrs[:], onesb_sb[:], r4[:],
                                start=(g == 0), stop=(g == ng - 1),
                            )
                    recip = sbp.tile([1, QB], F32R, tag="recip", bufs=2)
                    with nc.allow_low_precision(
                        reason="f32r reciprocal feeds the PE broadcast matmul"
                    ):
                        nc.vector.reciprocal(recip[:], psrs[:])
                    psb = ps.tile([128, QB], F32, tag="ps")
                    nc.tensor.matmul(
                        psb[:], ones_sb[0:1, :], recip[:],
                        start=True, stop=True,
                    )
                    psbsb = sbp.tile([128, QB], F32, tag="psbsb", bufs=1)
                    nc.vector.tensor_copy(psbsb[:], psb[:])
                    nc.vector.tensor_tensor(
                        at[:, hh, qs0:qs0 + QB], psat[:], psbsb[:], ALU.mult
                    )

                # C for this q-block: out[q, :] = sum_t at_t^T @ wo_t
                if "C" in phases:
                    for qs in range(QB // 128):
                        ob = sbp.tile([128, 1024], BF16, tag="ob", bufs=2)
                        for oc in range(h // 512):
                            pso = ps.tile([128, 512], F32, tag="ps")
                            for t in range(HPC):
                                nc.tensor.matmul(
                                    pso[:],
                                    at[:, t, qs0 + qs * 128:qs0 + qs * 128 + 128],
                                    wo_sb[:, t, oc * 512:oc * 512 + 512],
                                    start=(t == 0),
                                    stop=(t == HPC - 1),
                                )
                            nc.vector.tensor_copy(
                                ob[:, (oc % 2) * 512:(oc % 2) * 512 + 512],
                                pso[:],
                            )
                            if oc % 2 == 1:
                                nc.sync.dma_start(
                                    out_d[qs0 + qs * 128:qs0 + qs * 128 + 128,
                                          (oc // 2) * 1024:(oc // 2) * 1024 + 1024],
                                    ob[:],
                                )
                                if oc < 7:
                                    ob = sbp.tile([128, 1024], BF16, tag="ob",
                                                  bufs=2)
    return _finish(nc)


def _finish(nc):
    nc.compile()
    return nc


def make_in_maps(hidden_states, target_hidden, cos, sin, Wq, Wk, Wv, Wo):
    hidden_states = np.asarray(hidden_states, dtype=np.float32)
    target_hidden = np.asarray(target_hidden, dtype=np.float32)
    cos = np.asarray(cos, dtype=np.float32)
    sin = np.asarray(sin, dtype=np.float32)
    Wq = np.asarray(Wq, dtype=np.float32)
    Wk = np.asarray(Wk, dtype=np.float32)
    Wv = np.asarray(Wv, dtype=np.float32)
    Wo = np.asarray(Wo, dtype=np.float32)

    bf16 = ml_dtypes.bfloat16
    pi = head_perm()
    hT = np.ascontiguousarray(hidden_states[0].T).astype(bf16)
    tT = np.ascontiguousarray(target_hidden[0].T).astype(bf16)
    cP = np.ascontiguousarray(cos[0].T[pi, :]).astype(bf16)
    sP = sin[0].T[pi, :].copy()
    sgn = np.where((np.arange(128) % 32) < 16, -1.0, 1.0).astype(np.float32)
    sP = (sP * sgn[:, None]).astype(bf16)

    # permute head-dim columns of Wq/Wk within each 128-block
    def permute_cols(WT, nheads):
        # WT: [H, nheads*128] -> columns reordered by pi within each block
        cols = np.concatenate([b * 128 + pi for b in range(nheads)])
        return WT[:, cols]

    ident = np.eye(128, dtype=np.float32).astype(bf16)
    ones = np.ones((128, 128), dtype=np.float32)

    in_maps = []
    for c in range(NCORES):
        wq_c = np.ascontiguousarray(Wq[512 * c:512 * c + 512, :].T)
        wk_c = np.ascontiguousarray(Wk[128 * c:128 * c + 128, :].T)
        wv_c = np.ascontiguousarray(Wv[128 * c:128 * c + 128, :].T)
        in_maps.append({
            "hiddenT": hT,
            "targetT": tT,
            "cosP": cP,
            "sinP": sP,
            "wqT": np.ascontiguousarray(permute_cols(wq_c, HPC)).astype(bf16),
            "wkT": np.ascontiguousarray(permute_cols(wk_c, 1)).astype(bf16),
            "wvT": wv_c.astype(bf16),
            "woT": np.ascontiguousarray(Wo[:, 512 * c:512 * c + 512].T).astype(bf16),
            "ones": ones,
            "ones_bf": np.ones((128, 1), dtype=bf16),
            "identb": ident,
        })
    return in_maps


_CACHE = {}
LAST_EXEC_NS = None
TRACE = False


def kernel(hidden_states, target_hidden, cos, sin, Wq, Wk, Wv, Wo):
    global LAST_EXEC_NS
    if "nc" not in _CACHE:
        _CACHE["nc"] = build_program()
    nc = _CACHE["nc"]
    in_maps = make_in_maps(
        hidden_states, target_hidden, cos, sin, Wq, Wk, Wv, Wo
    )
    res = run_bass_kernel_spmd(
        nc, in_maps, list(range(NCORES)), trace=TRACE
    )
    LAST_EXEC_NS = res.exec_time_ns
    out = np.zeros((QL, H), dtype=np.float32)
    for r in res.results:
        out += r["out"].astype(np.float32)
    return out.reshape(1, QL, H)
